# revision 7
# baseline (speedup 1.0000x reference)
# GNN mean-aggregation kernel for Trainium2 (8 NeuronCores, SPMD).
#
# Computes: out[i] = (1/deg_i) * sum_{(i,j) in E} (x[j] @ W + b)
# using the algebraic identity  out = inv_deg * (A @ x) @ W + b*mask,
# so the dense linear layer runs on the 100k aggregated rows instead of
# per-edge features.
#
# Sharding: destination nodes (and their incoming edge rows -- `row` is
# sorted) are split contiguously across 8 cores; x and W are replicated,
# so no collectives are needed.
#
# Per-core pipeline:
#   1. dma_gather (GPSIMD SWDGE) fetches x[col] rows (512B each) from HBM
#      in 1024-index calls.  int16 gather indices only span 32k rows, so x
#      is addressed in 4 chunks of 25k rows and edges are host-binned by
#      (dest-tile, chunk), padded to a fixed per-bin column count so the
#      single SPMD instruction stream fits every core.
#   2. DVE builds one-hot segment matrices S^T[e,d] = (rel[e]==d) from
#      host-provided relative-dest values via tensor_tensor(is_equal).
#   3. PE accumulates AGG^T = sum_j M_j^T @ S^T_j in PSUM per 128-dest
#      tile, then OUT^T = W^T @ AGG^T + b (x) deg  (rank-1 bias matmul).
#   4. DVE scales by inv_deg along the dest axis; DMA writes OUT^T.
# Host post-processing transposes and concatenates the per-core outputs.

import math

import numpy as np

P = 128
F = 128


class _Cfg:
    def __init__(self, n_nodes, n_cores, n_chunks, group_tiles=8):
        self.NN = n_nodes
        self.NCORES = n_cores
        self.NDEST = n_nodes // n_cores
        self.NT = math.ceil(self.NDEST / P)
        self.NCH = n_chunks
        self.CH = math.ceil(n_nodes / n_chunks)
        assert self.CH <= 32768
        self.G = group_tiles


CFG = _Cfg(100000, 8, 4)

_BUILD_CACHE = {}


def _host_prep(cfg, x, row, col, W, b):
    NN, NCORES, NDEST, NT, NCH, CH = (
        cfg.NN, cfg.NCORES, cfg.NDEST, cfg.NT, cfg.NCH, cfg.CH)
    NE = row.shape[0]
    row = np.asarray(row).astype(np.int64)
    col = np.asarray(col).astype(np.int64)
    x = np.ascontiguousarray(np.asarray(x, dtype=np.float32))
    W = np.ascontiguousarray(np.asarray(W, dtype=np.float32))
    b = np.asarray(b, dtype=np.float32)

    deg = np.bincount(row, minlength=NN).astype(np.float32)
    invdeg = np.where(deg > 0, 1.0 / np.maximum(deg, 1.0), 0.0).astype(np.float32)

    core = row // NDEST
    r_in_core = row % NDEST
    tilei = r_in_core // P
    rel = (r_in_core % P).astype(np.float32)
    chunk = col // CH
    idx16 = (col % CH).astype(np.int16)

    bin_key = (core * NT + tilei) * NCH + chunk
    nbins = NCORES * NT * NCH
    counts = np.bincount(bin_key, minlength=nbins)
    C_sub = max(1, int(math.ceil(counts.max() / P)))
    SLOT = C_sub * P

    order = np.argsort(bin_key, kind="stable")
    sk = bin_key[order]
    starts = np.concatenate([[0], np.cumsum(counts)[:-1]])
    rank = np.arange(NE, dtype=np.int64) - starts[sk]
    pos = sk * SLOT + rank

    TOT = nbins * SLOT
    idx_pad = np.zeros(TOT, np.int16)
    rel_pad = np.full(TOT, -1.0, np.float32)
    idx_pad[pos] = idx16[order]
    rel_pad[pos] = rel[order]
    idx_pad = idx_pad.reshape(NCORES, NT, NCH, SLOT)
    rel_pad = rel_pad.reshape(NCORES, NT, NCH, C_sub, P)

    groups = [(t0, min(t0 + cfg.G, NT)) for t0 in range(0, NT, cfg.G)]

    iota2 = np.tile(np.arange(P, dtype=np.float32)[None, :], (P, 1))
    brow = b[None, :]

    in_maps = []
    for c in range(NCORES):
        # gather-call index stream: per (group, chunk), wrapped per <=1024-idx call
        wrapped_parts = []
        for (t0, t1) in groups:
            for ch in range(NCH):
                seq = idx_pad[c, t0:t1, ch].reshape(-1)
                for k0 in range(0, len(seq), 1024):
                    seg = seq[k0:k0 + 1024]
                    wrapped_parts.append(
                        np.tile(seg.reshape(-1, 16).T, (8, 1)))
        idx_t = np.concatenate(wrapped_parts, axis=1)

        rel_t = np.ascontiguousarray(
            rel_pad[c].transpose(3, 0, 1, 2).reshape(P, NT * NCH * C_sub))

        dsl = slice(c * NDEST, (c + 1) * NDEST)
        ivc = np.zeros(NT * P, np.float32)
        ivc[:NDEST] = invdeg[dsl]
        dgc = np.zeros(NT * P, np.float32)
        dgc[:NDEST] = deg[dsl]

        in_maps.append({
            "x": x,
            "idxs": np.ascontiguousarray(idx_t),
            "rel": rel_t,
            "invdeg": np.ascontiguousarray(np.tile(ivc[None, :], (P, 1))),
            "degr": dgc[None, :],
            "w": W,
            "brow": brow,
            "iota2": iota2,
        })
    return C_sub, in_maps


def _build(cfg, C_sub, repeat):
    import concourse.mybir as mybir
    import concourse.tile as tile
    from concourse import bacc

    f32 = mybir.dt.float32
    i16 = mybir.dt.int16
    eq = mybir.AluOpType.is_equal
    mult = mybir.AluOpType.mult

    NT, NCH, CH, G = cfg.NT, cfg.NCH, cfg.CH, cfg.G
    C_tot = NCH * C_sub
    IDXW = NT * C_tot * P // 16

    nc = bacc.Bacc("TRN2", debug=False)
    x_d = nc.dram_tensor("x", [cfg.NN, F], f32, kind="ExternalInput")
    idx_d = nc.dram_tensor("idxs", [P, IDXW], i16, kind="ExternalInput")
    rel_d = nc.dram_tensor("rel", [P, NT * C_tot], f32, kind="ExternalInput")
    invdeg_d = nc.dram_tensor("invdeg", [P, NT * P], f32, kind="ExternalInput")
    deg_d = nc.dram_tensor("degr", [1, NT * P], f32, kind="ExternalInput")
    w_d = nc.dram_tensor("w", [F, F], f32, kind="ExternalInput")
    b_d = nc.dram_tensor("brow", [1, F], f32, kind="ExternalInput")
    iota_d = nc.dram_tensor("iota2", [P, P], f32, kind="ExternalInput")
    out_d = nc.dram_tensor("outT", [P, NT * P], f32, kind="ExternalOutput")

    groups = [(t0, min(t0 + G, NT)) for t0 in range(0, NT, G)]
    x_ap = x_d.ap()

    with tile.TileContext(nc) as tc:
        with (
            tc.tile_pool(name="const", bufs=1) as constp,
            tc.tile_pool(name="reg", bufs=2) as regionp,
            tc.tile_pool(name="st", bufs=4) as stp,
            tc.tile_pool(name="idx", bufs=2) as idxp,
            tc.tile_pool(name="small", bufs=4) as smallp,
            tc.tile_pool(name="grp", bufs=2) as grpp,
            tc.tile_pool(name="acc", bufs=8, space="PSUM") as accp,
        ):
            w_sb = constp.tile([F, F], f32)
            nc.sync.dma_start(w_sb[:], w_d.ap())
            b_sb = constp.tile([1, F], f32)
            nc.sync.dma_start(b_sb[:], b_d.ap())
            iota_sb = constp.tile([P, P], f32)
            nc.sync.dma_start(iota_sb[:], iota_d.ap())
            rel_sb = constp.tile([P, NT * C_tot], f32)
            nc.sync.dma_start(rel_sb[:], rel_d.ap())

            def body(_iv=None):
                idx_off = 0
                for (t0, t1) in groups:
                    gt = t1 - t0
                    invdeg_g = grpp.tile([P, gt * P], f32, tag="invdeg")
                    nc.sync.dma_start(
                        invdeg_g[:], invdeg_d.ap()[:, t0 * P:t1 * P])
                    deg_g = grpp.tile([1, gt * P], f32, tag="deg")
                    nc.sync.dma_start(deg_g[:], deg_d.ap()[:, t0 * P:t1 * P])
                    accs = [
                        accp.tile([P, P], f32, tag="acc", name=f"acc{t0}_{k}")
                        for k in range(gt)
                    ]
                    for c in range(NCH):
                        ncols = gt * C_sub
                        reg = regionp.tile([P, ncols, P], f32, tag="reg")
                        idxt = idxp.tile([P, ncols * 8], i16, tag="idx")
                        nc.sync.dma_start(
                            idxt[:], idx_d.ap()[:, idx_off:idx_off + ncols * 8])
                        idx_off += ncols * 8
                        for k0 in range(0, ncols, 8):
                            kc = min(8, ncols - k0)
                            L = kc * P
                            nc.gpsimd.dma_gather(
                                out_ap=reg[:, k0:k0 + kc, :],
                                in_ap=x_ap[c * CH:min((c + 1) * CH, cfg.NN), :],
                                idxs_ap=idxt[:, k0 * 8:k0 * 8 + kc * 8],
                                num_idxs=L,
                                num_idxs_reg=L,
                                elem_size=F,
                            )
                        for ti in range(gt):
                            t = t0 + ti
                            st = stp.tile([P, C_sub, P], f32, tag="st")
                            rel_sl = rel_sb[:, (t * NCH + c) * C_sub:
                                            (t * NCH + c + 1) * C_sub]
                            nc.vector.tensor_tensor(
                                out=st[:],
                                in0=iota_sb[:].unsqueeze(1).to_broadcast(
                                    [P, C_sub, P]),
                                in1=rel_sl.to_broadcast([P, C_sub, P]),
                                op=eq,
                            )
                            accap = accs[ti][:]
                            for j in range(C_sub):
                                nc.tensor.matmul(
                                    out=accap,
                                    lhsT=reg[:, ti * C_sub + j, :],
                                    rhs=st[:, j, :],
                                    start=(c == 0 and j == 0),
                                    stop=(c == NCH - 1 and j == C_sub - 1),
                                )
                    for ti in range(gt):
                        t = t0 + ti
                        accap = accs[ti][:]
                        aggT = smallp.tile([P, P], f32, tag="agg")
                        nc.scalar.copy(aggT[:], accap)
                        # reuse the same PSUM bank for the output matmul
                        nc.tensor.matmul(out=accap, lhsT=w_sb[:], rhs=aggT[:],
                                         start=True, stop=False)
                        nc.tensor.matmul(out=accap, lhsT=b_sb[:1, :],
                                         rhs=deg_g[:1, ti * P:(ti + 1) * P],
                                         start=False, stop=True)
                        osb = smallp.tile([P, P], f32, tag="osb")
                        nc.vector.tensor_tensor(
                            out=osb[:], in0=accap,
                            in1=invdeg_g[:, ti * P:(ti + 1) * P], op=mult)
                        nc.sync.dma_start(
                            out_d.ap()[:, t * P:(t + 1) * P], osb[:])

            if repeat == 1:
                body()
            else:
                with tc.For_i(0, repeat, 1) as iv:
                    body(iv)

    nc.compile()
    return nc


def _run(cfg, x, row, col, W, b, repeat=1, core_ids=None):
    from concourse import bass_utils

    C_sub, in_maps = _host_prep(cfg, x, row, col, W, b)
    key = (cfg.NN, cfg.NCORES, C_sub, repeat)
    if key not in _BUILD_CACHE:
        _BUILD_CACHE[key] = _build(cfg, C_sub, repeat)
    nc = _BUILD_CACHE[key]
    if core_ids is None:
        core_ids = list(range(cfg.NCORES))
    res = bass_utils.run_bass_kernel_spmd(nc, in_maps, core_ids=core_ids)
    outs = []
    for c in range(len(core_ids)):
        outT = res.results[c]["outT"]
        outs.append(outT.T[:cfg.NDEST])
    return np.concatenate(outs, axis=0)


def kernel(x, row, col, W, b):
    return _run(CFG, x, row, col, W, b, repeat=1)


# revision 22
# speedup vs baseline: 2.9071x; 2.9071x over previous
# GNN mean-aggregation kernel for Trainium2 (8 NeuronCores, SPMD).
#
# Computes: out[i] = (1/deg_i) * sum_{(i,j) in E} (x[j] @ W + b)
# using the algebraic identity  out = inv_deg * (A @ x) @ W + b*mask,
# so the dense linear layer runs on the 100k aggregated rows instead of
# per-edge features.
#
# Sharding: destination nodes (and their incoming edge rows -- `row` is
# sorted) are split contiguously across 8 cores; x and W are replicated,
# so no collectives are needed.
#
# Per-core pipeline:
#   1. dma_gather (GPSIMD SWDGE) fetches x[col] rows (512B each) from HBM
#      in 1024-index calls.  int16 gather indices only span 32k rows, so x
#      is addressed in 4 chunks of 25k rows and edges are host-binned by
#      (dest-tile, chunk), padded to a fixed per-bin column count so the
#      single SPMD instruction stream fits every core.
#   2. DVE builds one-hot segment matrices S^T[e,d] = (rel[e]==d) from
#      host-provided relative-dest values via tensor_tensor(is_equal).
#   3. PE accumulates AGG^T = sum_j M_j^T @ S^T_j in PSUM per 128-dest
#      tile, then OUT^T = W^T @ AGG^T + b (x) deg  (rank-1 bias matmul).
#   4. DVE scales by inv_deg along the dest axis; DMA writes OUT^T.
# Host post-processing transposes and concatenates the per-core outputs.

import math

import numpy as np

P = 128
F = 128


class _Cfg:
    def __init__(self, n_nodes, n_cores, n_chunks, group_tiles=8):
        self.NN = n_nodes
        self.NCORES = n_cores
        self.NDEST = n_nodes // n_cores
        self.NT = math.ceil(self.NDEST / P)
        self.NCH = n_chunks
        self.CH = math.ceil(n_nodes / n_chunks)
        assert self.CH <= 32768
        self.G = group_tiles


CFG = _Cfg(100000, 8, 4)

_BUILD_CACHE = {}


def _host_prep(cfg, x, row, col, W, b):
    NN, NCORES, NDEST, NT, NCH, CH = (
        cfg.NN, cfg.NCORES, cfg.NDEST, cfg.NT, cfg.NCH, cfg.CH)
    NE = row.shape[0]
    row = np.asarray(row).astype(np.int64)
    col = np.asarray(col).astype(np.int64)
    x = np.ascontiguousarray(np.asarray(x, dtype=np.float32))
    W = np.ascontiguousarray(np.asarray(W, dtype=np.float32))
    b = np.asarray(b, dtype=np.float32)

    deg = np.bincount(row, minlength=NN).astype(np.float32)
    invdeg = np.where(deg > 0, 1.0 / np.maximum(deg, 1.0), 0.0).astype(np.float32)

    core = row // NDEST
    r_in_core = row % NDEST
    chunk = col // CH
    idx16 = (col % CH).astype(np.int16)

    # Natural (contiguous) dest->tile assignment unless some (tile, chunk)
    # bin would push C_sub above 9 columns; then greedily rebalance.
    nat_tile = r_in_core // P
    nat_key = (core * NT + nat_tile) * NCH + chunk
    nat_max = np.bincount(nat_key, minlength=NCORES * NT * NCH).max()
    if nat_max <= 9 * P:
        perm = np.tile(np.arange(NDEST, dtype=np.int64)[None, :], (NCORES, 1))
        tilei = nat_tile
        rel = (r_in_core % P).astype(np.float32)
        return _host_prep_finish(
            cfg, x, W, b, deg, invdeg, core, chunk, idx16, tilei, rel, perm)
    # perm[core, d_local] = permuted position (tile*128 + slot).
    perm = np.zeros((NCORES, NDEST), np.int64)
    for c in range(NCORES):
        cnt = np.zeros((NDEST, NCH), np.int32)
        np.add.at(cnt, (r_in_core[core == c], chunk[core == c]), 1)
        order_d = np.argsort(-cnt.max(axis=1), kind="stable")
        sums = np.zeros((NT, NCH), np.int32)
        counts = np.zeros(NT, np.int32)
        pos = np.empty(NDEST, np.int64)
        big = np.int32(1 << 30)
        for d in order_d:
            newmax = np.maximum(sums, cnt[d]).max(axis=1)
            t = int(np.argmin(np.where(counts < P, newmax, big)))
            pos[d] = t * P + counts[t]
            counts[t] += 1
            sums[t] += cnt[d]
        perm[c] = pos
    tilei = perm[core, r_in_core] // P
    rel = (perm[core, r_in_core] % P).astype(np.float32)
    return _host_prep_finish(
        cfg, x, W, b, deg, invdeg, core, chunk, idx16, tilei, rel, perm)


def _host_prep_finish(cfg, x, W, b, deg, invdeg, core, chunk, idx16,
                      tilei, rel, perm):
    NN, NCORES, NDEST, NT, NCH, CH = (
        cfg.NN, cfg.NCORES, cfg.NDEST, cfg.NT, cfg.NCH, cfg.CH)
    NE = core.shape[0]
    bin_key = (core * NT + tilei) * NCH + chunk
    nbins = NCORES * NT * NCH
    counts = np.bincount(bin_key, minlength=nbins)
    C_sub = max(1, int(math.ceil(counts.max() / P)))
    SLOT = C_sub * P

    order = np.argsort(bin_key, kind="stable")
    sk = bin_key[order]
    starts = np.concatenate([[0], np.cumsum(counts)[:-1]])
    rank = np.arange(NE, dtype=np.int64) - starts[sk]
    pos = sk * SLOT + rank

    TOT = nbins * SLOT
    idx_pad = np.zeros(TOT, np.int16)
    rel_pad = np.full(TOT, -1.0, np.float32)
    idx_pad[pos] = idx16[order]
    rel_pad[pos] = rel[order]
    idx_pad = idx_pad.reshape(NCORES, NT, NCH, SLOT)
    rel_pad = rel_pad.reshape(NCORES, NT, NCH, C_sub, P)

    groups = [(t0, min(t0 + cfg.G, NT)) for t0 in range(0, NT, cfg.G)]

    iota2 = np.tile(np.arange(P, dtype=np.float32)[None, :], (P, 1))
    brow = b[None, :]

    in_maps = []
    for c in range(NCORES):
        # gather-call index stream: per (group, chunk), wrapped per <=1024-idx call
        wrapped_parts = []
        for (t0, t1) in groups:
            for ch in range(NCH):
                seq = idx_pad[c, t0:t1, ch].reshape(-1)
                for k0 in range(0, len(seq), 1024):
                    seg = seq[k0:k0 + 1024]
                    wrapped_parts.append(
                        np.tile(seg.reshape(-1, 16).T, (8, 1)))
        idx_t = np.concatenate(wrapped_parts, axis=1)

        rel_t = np.ascontiguousarray(
            rel_pad[c].transpose(3, 0, 1, 2).reshape(P, NT * NCH * C_sub))

        dsl = slice(c * NDEST, (c + 1) * NDEST)
        ivc = np.zeros(NT * P, np.float32)
        ivc[perm[c]] = invdeg[dsl]
        dgc = np.zeros(NT * P, np.float32)
        dgc[perm[c]] = deg[dsl]

        in_maps.append({
            "x": x,
            "idxs": np.ascontiguousarray(idx_t),
            "rel": rel_t,
            "invdeg": np.ascontiguousarray(np.tile(ivc[None, :], (P, 1))),
            "degr": dgc[None, :],
            "w": W,
            "brow": brow,
            "iota2": iota2,
        })
    return C_sub, in_maps, perm


def _build(cfg, C_sub, repeat, parts=("gather", "onehot", "mm")):
    import concourse.mybir as mybir
    import concourse.tile as tile
    from concourse import bacc

    f32 = mybir.dt.float32
    i16 = mybir.dt.int16
    eq = mybir.AluOpType.is_equal
    mult = mybir.AluOpType.mult

    NT, NCH, CH, G = cfg.NT, cfg.NCH, cfg.CH, cfg.G
    C_tot = NCH * C_sub
    IDXW = NT * C_tot * P // 16

    nc = bacc.Bacc("TRN2", debug=False, num_swdge_queues=4)
    x_d = nc.dram_tensor("x", [cfg.NN, F], f32, kind="ExternalInput")
    idx_d = nc.dram_tensor("idxs", [P, IDXW], i16, kind="ExternalInput")
    rel_d = nc.dram_tensor("rel", [P, NT * C_tot], f32, kind="ExternalInput")
    invdeg_d = nc.dram_tensor("invdeg", [P, NT * P], f32, kind="ExternalInput")
    deg_d = nc.dram_tensor("degr", [1, NT * P], f32, kind="ExternalInput")
    w_d = nc.dram_tensor("w", [F, F], f32, kind="ExternalInput")
    b_d = nc.dram_tensor("brow", [1, F], f32, kind="ExternalInput")
    iota_d = nc.dram_tensor("iota2", [P, P], f32, kind="ExternalInput")
    out_d = nc.dram_tensor("outT", [P, NT * P], f32, kind="ExternalOutput")

    groups = [(t0, min(t0 + G, NT)) for t0 in range(0, NT, G)]
    x_ap = x_d.ap()

    with tile.TileContext(nc) as tc:
        with (
            tc.tile_pool(name="const", bufs=1) as constp,
            tc.tile_pool(name="reg", bufs=2) as regionp,
            tc.tile_pool(name="st", bufs=4) as stp,
            tc.tile_pool(name="idx", bufs=2) as idxp,
            tc.tile_pool(name="small", bufs=4) as smallp,
            tc.tile_pool(name="grp", bufs=2) as grpp,
            tc.tile_pool(name="acc", bufs=8, space="PSUM") as accp,
        ):
            w_sb = constp.tile([F, F], f32)
            nc.sync.dma_start(w_sb[:], w_d.ap())
            b_sb = constp.tile([1, F], f32)
            nc.sync.dma_start(b_sb[:], b_d.ap())
            iota_sb = constp.tile([P, P], f32)
            nc.sync.dma_start(iota_sb[:], iota_d.ap())
            rel_sb = constp.tile([P, NT * C_tot], f32)
            nc.sync.dma_start(rel_sb[:], rel_d.ap())

            def body(_iv=None):
                idx_off = 0
                qn = 0
                for (t0, t1) in groups:
                    gt = t1 - t0
                    invdeg_g = grpp.tile([P, gt * P], f32, tag="invdeg")
                    nc.sync.dma_start(
                        invdeg_g[:], invdeg_d.ap()[:, t0 * P:t1 * P])
                    deg_g = grpp.tile([1, gt * P], f32, tag="deg")
                    nc.sync.dma_start(deg_g[:], deg_d.ap()[:, t0 * P:t1 * P])
                    accs = [
                        accp.tile([P, P], f32, tag="acc", name=f"acc{t0}_{k}")
                        for k in range(gt)
                    ]
                    for c in range(NCH):
                        ncols = gt * C_sub
                        reg = regionp.tile([P, ncols, P], f32, tag="reg")
                        idxt = idxp.tile([P, ncols * 8], i16, tag="idx")
                        nc.sync.dma_start(
                            idxt[:], idx_d.ap()[:, idx_off:idx_off + ncols * 8])
                        idx_off += ncols * 8
                        for k0 in range(0, ncols, 8) if "gather" in parts else []:
                            kc = min(8, ncols - k0)
                            L = kc * P
                            nc.gpsimd.dma_gather(
                                out_ap=reg[:, k0:k0 + kc, :],
                                in_ap=x_ap[c * CH:min((c + 1) * CH, cfg.NN), :],
                                idxs_ap=idxt[:, k0 * 8:k0 * 8 + kc * 8],
                                num_idxs=L,
                                num_idxs_reg=L,
                                elem_size=F,
                                queue_num=qn % 4,
                            )
                            qn += 1
                        for ti in range(gt) if ("onehot" in parts or "mm" in parts) else []:
                            t = t0 + ti
                            st = stp.tile([P, C_sub, P], f32, tag="st")
                            rel_sl = rel_sb[:, (t * NCH + c) * C_sub:
                                            (t * NCH + c + 1) * C_sub]
                            if "onehot" in parts:
                                nc.vector.tensor_tensor(
                                    out=st[:],
                                    in0=iota_sb[:].unsqueeze(1).to_broadcast(
                                        [P, C_sub, P]),
                                    in1=rel_sl.to_broadcast([P, C_sub, P]),
                                    op=eq,
                                )
                            accap = accs[ti][:]
                            for j in range(C_sub) if "mm" in parts else []:
                                nc.tensor.matmul(
                                    out=accap,
                                    lhsT=reg[:, ti * C_sub + j, :],
                                    rhs=st[:, j, :],
                                    start=(c == 0 and j == 0),
                                    stop=(c == NCH - 1 and j == C_sub - 1),
                                )
                    for ti in range(gt) if "mm" in parts else []:
                        t = t0 + ti
                        accap = accs[ti][:]
                        aggT = smallp.tile([P, P], f32, tag="agg")
                        nc.scalar.copy(aggT[:], accap)
                        # reuse the same PSUM bank for the output matmul
                        nc.tensor.matmul(out=accap, lhsT=w_sb[:], rhs=aggT[:],
                                         start=True, stop=False)
                        nc.tensor.matmul(out=accap, lhsT=b_sb[:1, :],
                                         rhs=deg_g[:1, ti * P:(ti + 1) * P],
                                         start=False, stop=True)
                        osb = smallp.tile([P, P], f32, tag="osb")
                        nc.vector.tensor_tensor(
                            out=osb[:], in0=accap,
                            in1=invdeg_g[:, ti * P:(ti + 1) * P], op=mult)
                        nc.sync.dma_start(
                            out_d.ap()[:, t * P:(t + 1) * P], osb[:])

            if repeat == 1:
                body()
            else:
                with tc.For_i(0, repeat, 1) as iv:
                    body(iv)

    nc.compile()
    return nc


def _run(cfg, x, row, col, W, b, repeat=1, core_ids=None):
    from concourse import bass_utils

    C_sub, in_maps, perm = _host_prep(cfg, x, row, col, W, b)
    key = (cfg.NN, cfg.NCORES, C_sub, repeat)
    if key not in _BUILD_CACHE:
        _BUILD_CACHE[key] = _build(cfg, C_sub, repeat)
    nc = _BUILD_CACHE[key]
    if core_ids is None:
        core_ids = list(range(cfg.NCORES))
    res = bass_utils.run_bass_kernel_spmd(nc, in_maps, core_ids=core_ids)
    outs = []
    for c in range(len(core_ids)):
        outT = res.results[c]["outT"]
        outs.append(outT.T[perm[c]])
    return np.concatenate(outs, axis=0)


def kernel(x, row, col, W, b):
    return _run(CFG, x, row, col, W, b, repeat=1)
